# revision 8
# baseline (speedup 1.0000x reference)
"""MultiHeadEMA on 8 Trainium2 NeuronCores.

Strategy
--------
Channel-sharded: embed_dim=1024 -> 8 slices of 128 channels (= SBUF
partitions), one per core. The reference's FFT conv is exactly an order-2 IIR
    y_n[l] = q_n y_n[l-1] + x[l],   out = silu(c0 y0 + c1 y1 + omega x)
computed with `tensor_tensor_scan` on the vector engine.

The DVE scan runs at ~2.1 cyc/elem, so the recurrence is decimated by 4:
    Y_n[j] = y_n[4j] satisfies  Y_n[j] = q_n^4 Y_n[j-1] + u_n[j]
    u_n[j] = x[4j] + q_n x[4j-1] + q_n^2 x[4j-2] + q_n^3 x[4j-3]
u_n is built by accumulating diagonal matmuls (tensor engine, bf16) into PSUM
from stride-4 views of x; the scan reads u straight from PSUM at 1/4 length.
The intermediate phases y[4j+r] are never materialized: the outputs
    pre_r = c0 y0[4j+r] + c1 y1[4j+r] + w x[4j+r]
expand into diagonal matmuls over (Y0, Y1, stride-4 x views) with per-channel
coefficients (c_n q_n^r etc.), accumulated in PSUM, then one Silu activation
per 1024 columns evacuates PSUM -> SBUF with an interleaved stride-4 write.
Interior is bf16 (fp32 PSUM accumulation, fp32 scan state, exact fp32 decay
factors). Keeping the tensor engine densely fed matters: it up-clocks from
1.2 to 2.4 GHz only under sustained back-to-back matmul load.

Host side only reshapes/casts: per-core x slice -> [128, B, L] bf16; output
bf16 -> fp32.
"""

import numpy as np
import ml_dtypes

import concourse.bass as bass
import concourse.bacc as bacc
import concourse.tile as tile
from concourse import mybir
from concourse.bass_utils import run_bass_kernel_spmd

SEQ_LEN, BSZ, EMBED_DIM, NDIM = 4096, 4, 1024, 2
N_CORES = 8
D_PER = EMBED_DIM // N_CORES  # 128 channels/core = full SBUF partitions
SCALE = (1.0 / NDIM) ** 0.5
DEC = 4                   # decimation factor
J = SEQ_LEN // DEC        # decimated length 1024
CH = 512                  # matmul chunk (one fp32 PSUM bank)
NG = J // CH              # j-groups per slab (2)
F32 = mybir.dt.float32
BF16 = mybir.dt.bfloat16
AF = mybir.ActivationFunctionType
ALU = mybir.AluOpType


def build_bass():
    nc = bacc.Bacc(name="multihead_ema")
    x = nc.dram_tensor("x", [D_PER, BSZ, SEQ_LEN], BF16, kind="ExternalInput")
    # coef columns: [delta0, delta1, alpha0, alpha1, beta0, beta1, gamma0, gamma1, omega]
    coef = nc.dram_tensor("coef", [D_PER, 9], F32, kind="ExternalInput")
    eye = nc.dram_tensor("eye", [D_PER, D_PER], BF16, kind="ExternalInput")
    out = nc.dram_tensor("out", [D_PER, BSZ, SEQ_LEN], BF16, kind="ExternalOutput")

    with tile.TileContext(nc) as tc:
        with (
            tc.tile_pool(name="const", bufs=1) as const,
            tc.tile_pool(name="xp", bufs=3) as xp,
            tc.tile_pool(name="yp", bufs=2) as yp,
            tc.tile_pool(name="op", bufs=3) as op,
            tc.tile_pool(name="psu", bufs=2, space="PSUM") as psu,
            tc.tile_pool(name="psc", bufs=2, space="PSUM") as psc,
        ):
            csb = const.tile([D_PER, 9], F32)
            nc.sync.dma_start(out=csb[:, :], in_=coef[:, :])
            eyesb = const.tile([D_PER, D_PER], BF16)
            nc.sync.dma_start(out=eyesb[:, :], in_=eye[:, :])

            # --- per-channel coefficients ([128, 1/2] fp32, trivial)
            sig = const.tile([D_PER, 4], F32)  # [p0, p1, sa0, sa1]
            nc.scalar.activation(out=sig[:, :], in_=csb[:, 0:4], func=AF.Sigmoid)
            pq = const.tile([D_PER, NDIM], F32)
            nc.vector.tensor_mul(out=pq[:, :], in0=sig[:, 0:2], in1=sig[:, 2:4])
            q = const.tile([D_PER, NDIM], F32)  # q = 1 - p*sigmoid(alpha)
            nc.scalar.activation(out=q[:, :], in_=pq[:, :], func=AF.Copy,
                                 scale=-1.0, bias=1.0)
            q2 = const.tile([D_PER, NDIM], F32)
            nc.vector.tensor_mul(out=q2[:, :], in0=q[:, :], in1=q[:, :])
            q3 = const.tile([D_PER, NDIM], F32)
            nc.vector.tensor_mul(out=q3[:, :], in0=q2[:, :], in1=q[:, :])
            q4 = const.tile([D_PER, NDIM], F32)
            nc.vector.tensor_mul(out=q4[:, :], in0=q2[:, :], in1=q2[:, :])
            c1t = const.tile([D_PER, NDIM], F32)
            nc.vector.tensor_mul(out=c1t[:, :], in0=sig[:, 0:2], in1=csb[:, 4:6])
            c2t = const.tile([D_PER, NDIM], F32)
            nc.vector.tensor_mul(out=c2t[:, :], in0=c1t[:, :], in1=csb[:, 6:8])
            cc = const.tile([D_PER, NDIM], F32)  # c_n = p beta gamma scale
            nc.scalar.mul(out=cc[:, :], in_=c2t[:, :], mul=SCALE)
            cq = const.tile([D_PER, NDIM], F32)   # c_n q_n
            nc.vector.tensor_mul(out=cq[:, :], in0=cc[:, :], in1=q[:, :])
            cq2 = const.tile([D_PER, NDIM], F32)  # c_n q_n^2
            nc.vector.tensor_mul(out=cq2[:, :], in0=cc[:, :], in1=q2[:, :])
            cq3 = const.tile([D_PER, NDIM], F32)  # c_n q_n^3
            nc.vector.tensor_mul(out=cq3[:, :], in0=cc[:, :], in1=q3[:, :])
            csum = const.tile([D_PER, 1], F32)    # c0 + c1 + w
            nc.vector.tensor_add(out=csum[:, :], in0=cc[:, 0:1], in1=cc[:, 1:2])
            nc.vector.tensor_add(out=csum[:, :], in0=csum[:, :], in1=csb[:, 8:9])
            cqs = const.tile([D_PER, 1], F32)     # c0 q0 + c1 q1
            nc.vector.tensor_add(out=cqs[:, :], in0=cq[:, 0:1], in1=cq[:, 1:2])
            cq2s = const.tile([D_PER, 1], F32)    # c0 q0^2 + c1 q1^2
            nc.vector.tensor_add(out=cq2s[:, :], in0=cq2[:, 0:1], in1=cq2[:, 1:2])

            # --- bf16 diagonal weight matrices
            _dn = [0]

            def diag(scalar_ap):
                _dn[0] += 1
                t = const.tile([D_PER, D_PER], BF16, tag=f"diag{_dn[0]}")
                nc.vector.tensor_scalar_mul(out=t[:, :], in0=eyesb[:, :],
                                            scalar1=scalar_ap)
                return t

            w_q = [[diag(t[:, n : n + 1]) for n in range(NDIM)] for t in (q, q2, q3)]
            w_cy = [[diag(t[:, n : n + 1]) for n in range(NDIM)]
                    for t in (cc, cq, cq2, cq3)]  # Y-term weights for r=0..3
            w_w = diag(csb[:, 8:9])    # x term of pre_0
            w_cw = diag(csum[:, 0:1])  # x_pr self term, r>=1
            w_cqs = diag(cqs[:, 0:1])
            w_cq2s = diag(cq2s[:, 0:1])

            q4b = [q4[:, n : n + 1].to_broadcast([D_PER, J]) for n in range(NDIM)]

            for b in range(BSZ):
                xb = xp.tile([D_PER, SEQ_LEN], BF16)
                nc.sync.dma_start(out=xb[:, :], in_=x[:, b, :])

                # stride-4 phase views: xph(r, g) = x[4j+r] for j in group g
                def xph(r, g, shift=0):
                    # columns 4*(512g + j) + r - 4*shift for j = 0..511
                    base = 4 * CH * g + r - 4 * shift
                    if base < 0:
                        # drop j=0 (x index < 0): start at j=1
                        return xb[:, base + 4 : base + 4 * (CH - 1) + 1 : 4]
                    return xb[:, base : base + 4 * (CH - 1) + 1 : 4]

                # --- u_n in PSUM, Y_n = scan(q_n^4, u_n)
                Y = []
                for n in range(NDIM):
                    pu = psu.tile([D_PER, J], F32, tag="u")
                    for g in range(NG):
                        s = bass.ts(g, CH)
                        nc.tensor.matmul(pu[:, s], eyesb[:, :], xph(0, g),
                                         start=True, stop=False)
                        for k in range(1, 4):  # q^k * x[4j-k]
                            rhs = xph(4 - k, g, shift=1)
                            tgt = pu[:, g * CH + 1 : (g + 1) * CH] if g == 0 else pu[:, s]
                            last = k == 3
                            nc.tensor.matmul(tgt, w_q[k - 1][n][:, :], rhs,
                                             start=False, stop=last)
                    yn = yp.tile([D_PER, J], BF16, tag=f"y{n}")
                    nc.vector.tensor_tensor_scan(
                        out=yn[:, :], data0=q4b[n], data1=pu[:, :],
                        initial=0.0, op0=ALU.mult, op1=ALU.add,
                    )
                    Y.append(yn)

                # --- outputs: pre_r accumulated in PSUM, silu evacuates
                ob = op.tile([D_PER, SEQ_LEN], BF16)
                for g in range(NG):
                    s = bass.ts(g, CH)
                    for pair in range(2):  # phases (0,1) then (2,3)
                        pt = psc.tile([D_PER, 2 * CH], F32, tag="cmb")
                        for h in range(2):
                            r = 2 * pair + h
                            tgt = pt[:, bass.ts(h, CH)]
                            nc.tensor.matmul(tgt, w_cy[r][0][:, :], Y[0][:, s],
                                             start=True, stop=False)
                            nc.tensor.matmul(tgt, w_cy[r][1][:, :], Y[1][:, s],
                                             start=False, stop=False)
                            # x terms
                            xw = [(w_w if r == 0 else w_cw, r)]
                            if r == 2:
                                xw.append((w_cqs, 1))
                            elif r == 3:
                                xw.append((w_cqs, 2))
                                xw.append((w_cq2s, 1))
                            for i, (wt, rr) in enumerate(xw):
                                nc.tensor.matmul(tgt, wt[:, :], xph(rr, g),
                                                 start=False, stop=(i == len(xw) - 1))
                        # silu: pt[:, h*512 + k] -> ob col 4*(512g + k) + 2*pair + h
                        in_ap = pt[:, :].rearrange("p (h k) -> p k h", h=2)
                        oap = ob[:, :]
                        out_ap = bass.AP(
                            tensor=oap.tensor,
                            offset=oap.offset + 4 * CH * g + 2 * pair,
                            ap=[list(oap.ap[0]), [4, CH], [1, 2]],
                        )
                        nc.scalar.activation(out=out_ap, in_=in_ap, func=AF.Silu)
                nc.sync.dma_start(out=out[:, b, :], in_=ob[:, :])

    nc.compile()
    return nc


_CACHE: dict = {}


def _get_nc():
    if "nc" not in _CACHE:
        _CACHE["nc"] = build_bass()
    return _CACHE["nc"]


def make_in_maps(inputs):
    x = np.asarray(inputs["x"], np.float32)
    delta = np.asarray(inputs["delta"], np.float32).reshape(EMBED_DIM, NDIM)
    alpha = np.asarray(inputs["alpha"], np.float32).reshape(EMBED_DIM, NDIM)
    beta = np.asarray(inputs["beta"], np.float32).reshape(EMBED_DIM, NDIM)
    gamma = np.asarray(inputs["gamma"], np.float32).reshape(EMBED_DIM, NDIM)
    omega = np.asarray(inputs["omega"], np.float32).reshape(EMBED_DIM, 1)
    coef_full = np.concatenate([delta, alpha, beta, gamma, omega], axis=1)
    eye = np.eye(D_PER, dtype=ml_dtypes.bfloat16)
    in_maps = []
    for c in range(N_CORES):
        sl = slice(c * D_PER, (c + 1) * D_PER)
        xc = np.ascontiguousarray(
            x[:, :, sl].transpose(2, 1, 0).astype(ml_dtypes.bfloat16)
        )
        in_maps.append(
            {"x": xc, "coef": np.ascontiguousarray(coef_full[sl]), "eye": eye}
        )
    return in_maps


def gather_out(results):
    out = np.empty((SEQ_LEN, BSZ, EMBED_DIM), np.float32)
    for c in range(N_CORES):
        out[:, :, c * D_PER : (c + 1) * D_PER] = (
            results[c]["out"].astype(np.float32).transpose(2, 1, 0)
        )
    return out


def _run(inputs, **kwargs):
    nc = _get_nc()
    in_maps = make_in_maps(inputs)
    res = run_bass_kernel_spmd(nc, in_maps, core_ids=list(range(N_CORES)), **kwargs)
    return gather_out(res.results), res


def kernel(**inputs) -> np.ndarray:
    out, _ = _run(inputs)
    return out
